# revision 1
# baseline (speedup 1.0000x reference)
"""Trainium2 Bass kernel for nn_Attention (B=4, N=2048, C=768, H=12).

Sharding: 8 cores = 4 batches x 2 head-groups (6 heads = 3 pairs each),
Megatron-style tensor parallel on heads. Each core computes qkv for its head
slice, attention, and the head-group's projection partial out [2048, 768];
the host sums the 2 group partials per batch and adds the bias.

Design (cost-model driven; matmul cost = out_free_size x 1 cycle/row for f16):
  - All matmul operands fp16 (host-cast inputs); PSUM accumulation fp32.
  - x resident in SBUF (one [128, CO, 2048] tile), read from DRAM once via a
    handful of merged DMAs (SP issues DMAs serially at ~650ns each).
  - S^T tiles [128 j, 1024] = two heads x 512 i, exp'd on the Act engine
    straight out of PSUM into fp16 ptiles. The Act engine carries ~200us of
    exp work - the second-largest engine load after the PE (~188us).
  - SCHRAUD tiles' exp runs instead on the DVE as a Schraudolph fast exp
    (fp16 bits of e^x = int16(x*2^10/ln2 + 15*2^10 - 45), one tensor_scalar
    with int16-convert out, bitcast back to f16). Measured end-to-end rel
    err 1.0e-2 vs the fp32 reference (tolerance 2e-2); exact-exp-only runs
    at 1.0e-3 (KERNEL_SCHRAUD=0) but ~11us slower.
  - PV in [i, d] layout: po[i, 65] += ptile[j, i-block].T @ v[j, d|ones]
    (free size 65 instead of 512 -> half the PE cost of a [d, i] PV).
    Column 64 of v is ones so row sums Z accumulate alongside. One PSUM
    accumulation group per po bank (start zeroes the whole 2KB region).
  - PV for window (pair, i4) is deferred one window: it fills PE slack while
    the next window's exp stream keeps the Act/PE pipeline saturated.
  - Normalize: 1/Z per-partition (token) via DVE reciprocal + tensor_tensor
    broadcast multiply -> [i, d] fp16; PE-transpose (identity) -> ot [d, i];
    projection accumulates all 3 pairs into one PSUM tile per token block;
    output partials are written f16 (summed f32 on host).
  - qkv / v / proj / transpose work is emitted via a deadline+credit filler
    queue into the attention windows' PE slack; v production is split per
    head-pair so each pair's v lands in its own (quiet) window span.
  - Tiny pre-warm matmuls pin pe_busy_start through the DMA lead-in so real
    matmuls run at full speed (the p-state ramp costs 2x for the first 3us).
PSUM budget (8 banks): stm 2x2 + po 2x1 + mm 2x1.
"""

import os
import sys
from contextlib import ExitStack

if "/opt/trn_rl_repo" not in sys.path:
    sys.path.insert(0, "/opt/trn_rl_repo")

import numpy as np

import concourse.bass as bass
import concourse.mybir as mybir
import concourse.tile as tile
from concourse import bass_utils

F32 = mybir.dt.float32
F16 = mybir.dt.float16
I16 = mybir.dt.int16

B, N, C = 4, 2048, 768
NH, D = 12, 64
SCALE = D ** -0.5
HPC = 6                 # heads per core
PAIRS = 3
P = 128
CO = C // P             # 6 contraction chunks
NO = N // P             # 16 token chunks of 128
NI4 = 4                 # i-chunks of 512
NCORES = 8
F = HPC * D             # 384

# Schraudolph fast-exp offload to DVE: number of (window, j) tiles rerouted.
# 0 disables. Tiles are taken from late windows (see _schraud_set).
SCHRAUD = int(os.environ.get("KERNEL_SCHRAUD", "50"))
# fp16 bits of e^x ~ int16(x * 2^10/ln2 + 15*2^10 - shift)
SCHRAUD_A = float(2 ** 10 / np.log(2.0))
SCHRAUD_B = float(os.environ.get("KERNEL_SCHRAUD_B", str(15 * 2 ** 10 - 45)))


def _split_multiwaits(nc):
    """This container's walrus accepts at most ONE sync-wait per instruction.

    Split any instruction carrying N>1 waits into (N-1) single-wait NOPs on
    the same engine queue placed immediately before it (engine queues are
    FIFO, so the semantics are identical)."""
    ctr = 0
    for f in nc.m.functions:
        for blk in f.blocks:
            insts = blk.instructions
            out = []
            changed = False
            for ins in insts:
                si = ins.sync_info
                if si is not None and len(si.on_wait) > 1:
                    changed = True
                    waits = list(si.on_wait)
                    for ww in waits[:-1]:
                        nop = mybir.InstNoOp(name=f"zzsplitw_{ctr}", ins=[], outs=[])
                        ctr += 1
                        nop.engine = ins.engine
                        nop.sync_info = mybir.SyncInfo(on_wait=[ww], on_update=[])
                        out.append(nop)
                    ins.sync_info = mybir.SyncInfo(
                        on_wait=[waits[-1]], on_update=list(si.on_update)
                    )
                out.append(ins)
            if changed:
                blk.instructions = out
    return nc


def _schraud_set():
    """(widx, j) tiles whose exp runs as DVE Schraudolph instead of Act exp.

    Spread across late windows, avoiding j==0/15 (PSUM start/stop edges are
    innocuous but keep the pattern simple) - per output row only a slice of
    attention positions is approximated, keeping the error ~ the measured
    one-pair level (7.5e-3)."""
    s = set()
    if SCHRAUD <= 0:
        return s
    picks = []
    for widx in range(11, -1, -1):
        for j in (5, 11, 8, 2, 14):
            picks.append((widx, j))
    for t in picks[:SCHRAUD]:
        s.add(t)
    return s


def _emit(nc, tc, ctx):
    xcd = nc.dram_tensor("xcd", [CO, P, N], F16, kind="ExternalInput").ap()
    wqd = nc.dram_tensor("wqd", [CO, P, 3 * F], F16, kind="ExternalInput").ap()
    wpd = nc.dram_tensor("wpd", [PAIRS, P, C], F16, kind="ExternalInput").ap()
    identd = nc.dram_tensor("identd", [P, P], F16, kind="ExternalInput").ap()
    outd = nc.dram_tensor("outd", [N, C], F16, kind="ExternalOutput").ap()

    persist = ctx.enter_context(tc.tile_pool(name="persist", bufs=1))
    xs_all = persist.tile([P, CO, N], F16, tag="xs")
    ws_all = persist.tile([P, CO, 3 * F], F16, tag="ws")
    xs = [xs_all[:, co, :] for co in range(CO)]
    ws = [ws_all[:, co, :] for co in range(CO)]
    qs = [persist.tile([P, N], F16, tag=f"qs{pr}", name=f"qs{pr}")
          for pr in range(PAIRS)]
    ks_ = [persist.tile([P, N], F16, tag=f"ks{pr}", name=f"ks{pr}")
           for pr in range(PAIRS)]
    vs = [persist.tile([P, HPC, D + 1], F16, tag=f"vs{no}", name=f"vs{no}")
          for no in range(NO)]
    ots = [persist.tile([P, N], F16, tag=f"ots{pr}", name=f"ots{pr}")
           for pr in range(PAIRS)]
    wps = persist.tile([P, PAIRS, C], F16, tag="wps")
    ident = persist.tile([P, P], F16, tag="ident")

    # exp table warm: pulls the ACT table load into the DMA lead-in window
    warm = persist.tile([P, 8], F32, tag="warm")
    nc.vector.memset(warm, 1.0)
    expwarm = persist.tile([P, 8], F32, tag="expwarm")
    nc.scalar.activation(
        out=expwarm, in_=warm, func=mybir.ActivationFunctionType.Exp, scale=1.0
    )


    # Input DMAs. Order = SP-queue order (650ns serial issue per DMA), so
    # merge aggressively: pair-0 weights in one DMA, then per-co x chunks
    # (kept separate so the first matmuls start as soon as their chunk
    # lands), then one merged DMA per remaining region.
    xcd_p = xcd.rearrange("c p n -> p c n")
    wqd_p = wqd.rearrange("c p f -> p c f")
    nc.sync.dma_start(out=ws_all[:, :, 0:256], in_=wqd_p[:, :, 0:256])
    for ch in range(3):
        nc.sync.dma_start(
            out=xs_all[:, 2 * ch:2 * ch + 2, 0:512],
            in_=xcd_p[:, 2 * ch:2 * ch + 2, 0:512],
        )
    for n4 in (1, 2, 3):
        nc.sync.dma_start(
            out=xs_all[:, :, n4 * 512:(n4 + 1) * 512],
            in_=xcd_p[:, :, n4 * 512:(n4 + 1) * 512],
        )
    nc.sync.dma_start(out=ws_all[:, :, 256:1152], in_=wqd_p[:, :, 256:1152])
    nc.sync.dma_start(out=ident, in_=identd)
    nc.sync.dma_start(out=wps, in_=wpd.rearrange("r p c -> p r c"))

    stp = ctx.enter_context(tc.tile_pool(name="stp", bufs=2, space="PSUM"))
    pop_ = ctx.enter_context(tc.tile_pool(name="pop", bufs=2, space="PSUM"))
    mmp = ctx.enter_context(tc.tile_pool(name="mmp", bufs=2, space="PSUM"))
    ptp = ctx.enter_context(tc.tile_pool(name="ptp", bufs=28))
    otnp = ctx.enter_context(tc.tile_pool(name="otnp", bufs=3))
    rzp = ctx.enter_context(tc.tile_pool(name="rzp", bufs=4))
    osbp = ctx.enter_context(tc.tile_pool(name="osbp", bufs=6))

    # PE pre-warm: tiny dummy matmuls keep the PE continuously busy through
    # the DMA lead-in so the p-state ramp (2x slower cycles for the first
    # 3us of busy time) is spent before the first real matmul. They rotate
    # through the mm slots ahead of any real user, costing no extra banks.
    for i in range(26):
        pw = mmp.tile([8, 8], F32, tag="mm", name=f"pw{i}")
        nc.tensor.matmul(pw, warm[:, 0:8], warm[:, 0:8], start=True, stop=True)

    # vs ones-columns: needed only by the first PV (slot 16); emitted here so
    # the DVE queue serves the front q/k evacuations first
    for no in range(NO):
        nc.vector.memset(vs[no][:, :, D:D + 1], 1.0)

    def qk_step(pr, qk, n4, co, box):
        if co == 0:
            box["pq"] = mmp.tile([P, 512], F32, tag="mm", name=f"pq_{pr}{qk}{n4}")
        col = pr * 256 + qk * 128
        nc.tensor.matmul(
            box["pq"],
            ws[co][:, col:col + 128],
            xs[co][:, n4 * 512:(n4 + 1) * 512],
            start=(co == 0),
            stop=(co == CO - 1),
        )
        if co == CO - 1:
            dst = (qs if qk == 0 else ks_)[pr]
            nc.vector.tensor_copy(out=dst[:, n4 * 512:(n4 + 1) * 512], in_=box["pq"])

    def emit_qk(pr, qk, n4):
        box = {}
        for co in range(CO):
            qk_step(pr, qk, n4, co, box)

    def qk_step_rest(pr, co, box):
        # k chunk n4=0 columns 128:512 (first 128 handled by the front split)
        if co == 0:
            box["pq"] = mmp.tile([P, 384], F32, tag="mm", name=f"pkr_{pr}")
        nc.tensor.matmul(
            box["pq"],
            ws[co][:, pr * 256 + 128:pr * 256 + 256],
            xs[co][:, 128:512],
            start=(co == 0),
            stop=(co == CO - 1),
        )
        if co == CO - 1:
            nc.vector.tensor_copy(out=ks_[pr][:, 128:512], in_=box["pq"])

    def v_step(p3, no, co, box):
        if co == 0:
            box["pv"] = mmp.tile([P, 128], F32, tag="mm", name=f"pv_{p3}_{no}")
        nc.tensor.matmul(
            box["pv"],
            xs[co][:, no * 128:(no + 1) * 128],
            ws[co][:, 768 + p3 * 128:768 + (p3 + 1) * 128],
            start=(co == 0),
            stop=(co == CO - 1),
        )
        if co == CO - 1:
            nc.vector.tensor_copy(
                out=vs[no][:, 2 * p3:2 * p3 + 2, 0:D],
                in_=box["pv"].rearrange("p (h d) -> p h d", h=2),
            )

    def add_qk_fillers(pr, qk, n4, dl):
        box = {}
        for co in range(CO):
            add_filler(215, dl, lambda pr=pr, qk=qk, n4=n4, co=co, box=box:
                       qk_step(pr, qk, n4, co, box))

    def add_v_fillers(p3, no, dl):
        box = {}
        for co in range(CO):
            add_filler(120, dl, lambda p3=p3, no=no, co=co, box=box:
                       v_step(p3, no, co, box))

    def emit_proj(no):
        osb = osbp.tile([P, C], F16, tag="osb", name=f"osb_{no}")
        for half in range(2):
            # tail groups (no>=12) alternate onto the stm slots - idle once
            # the last window's exps are consumed - doubling the effective
            # psum depth of the proj stream
            pool = stp if (no >= 11 and (no + half) % 2) else mmp
            tag = "st" if pool is stp else "mm"
            pp = pool.tile([P, 384], F32, tag=tag, name=f"pp_{no}_{half}")
            for p3 in range(PAIRS):
                nc.tensor.matmul(
                    pp,
                    ots[p3][:, no * 128:(no + 1) * 128],
                    wps[:, p3, half * 384:(half + 1) * 384],
                    start=(p3 == 0),
                    stop=(p3 == PAIRS - 1),
                )
            nc.vector.tensor_copy(out=osb[:, half * 384:(half + 1) * 384], in_=pp)
        nc.sync.dma_start(out=outd[no * 128:(no + 1) * 128, :], in_=osb)

    # -- filler queue: (cost_ns, deadline slot or None, fn) --
    # Slots are linearized (widx*16 + j); pop_fillers(s) runs at the END of
    # slot s, so a filler a consumer at slot s depends on must carry deadline
    # <= s-1 (emission order defines both engine-queue order and the tile
    # dependency graph - a filler emitted after its consumer is a race).
    fillers = []
    fidx = [0]
    credit = [0.0]

    def add_filler(cost, dl, fn):
        fillers.append((cost, dl, fn))

    def pop_fillers(now):
        while fidx[0] < len(fillers):
            cost, dl, fn = fillers[fidx[0]]
            due = dl is not None and dl <= now
            if not due:
                later_due = any(
                    d is not None and d <= now for _, d, _ in fillers[fidx[0]:]
                )
                if not later_due and credit[0] < cost:
                    break
            fn()
            credit[0] = max(credit[0] - cost, -1200.0)
            fidx[0] += 1

    def flush_fillers():
        while fidx[0] < len(fillers):
            _, _, fn = fillers[fidx[0]]
            fn()
            fidx[0] += 1

    schraud = _schraud_set()
    ptiles = {}

    def emit_drain(w, po_a, po_b):
        ppr, pi4 = w
        for half, po in ((0, po_a), (1, po_b)):
            po_r = po.rearrange("p (r z) -> p r z", z=65)
            rz = rzp.tile([P, 4], F32, tag="rz", name=f"rz_{ppr}{pi4}{half}")
            nc.vector.reciprocal(out=rz, in_=po_r[:, :, 64])
            for ibh in range(2):
                ib = half * 2 + ibh
                otn = otnp.tile([P, P], F16, tag="otn", name=f"otn_{ppr}{pi4}{ib}")
                nc.vector.tensor_tensor(
                    out=otn,
                    in0=po_r[:, 2 * ibh:2 * ibh + 2, 0:64],
                    in1=rz[:, 2 * ibh:2 * ibh + 2].rearrange(
                        "p (r one) -> p r one", one=1
                    ).to_broadcast([P, 2, 64]),
                    op=mybir.AluOpType.mult,
                )
                tp = mmp.tile([P, P], F16, tag="mm", name=f"tp_{ppr}{pi4}{ib}")
                nc.tensor.transpose(tp, otn, ident)
                blk = pi4 * 4 + ib
                nc.vector.tensor_copy(
                    out=ots[ppr][:, blk * 128:(blk + 1) * 128], in_=tp
                )

        if ppr == PAIRS - 1:
            for no in range(4 * pi4, 4 * pi4 + 4):
                add_filler(1000, None, lambda no=no: emit_proj(no))

    def window(widx, cur, prev, prev_pos):
        if prev is not None:
            po_a = pop_.tile([P, 260], F32, tag="po", name=f"poa_{prev[0]}{prev[1]}")
            po_b = pop_.tile([P, 260], F32, tag="po", name=f"pob_{prev[0]}{prev[1]}")
        if cur is not None:
            pr, i4 = cur
            ptiles[cur] = [None] * NO
        for j in range(NO):
            if cur is not None:
                stm = stp.tile([P, 1024], F32, tag="st", name=f"st_{pr}{i4}{j}")
                nc.tensor.matmul(
                    stm[:, 0:512],
                    ks_[pr][0:64, j * 128:(j + 1) * 128],
                    qs[pr][0:64, i4 * 512:(i4 + 1) * 512],
                    start=True, stop=True,
                )
                nc.tensor.matmul(
                    stm[:, 512:1024],
                    ks_[pr][64:128, j * 128:(j + 1) * 128],
                    qs[pr][64:128, i4 * 512:(i4 + 1) * 512],
                    start=True, stop=True,
                )
                pt = ptp.tile([P, 1024], F16, tag="pt", name=f"ptile_{pr}{i4}{j}")
                ptiles[cur][j] = pt
                if (widx, j) in schraud:
                    # fast exp on DVE: fp16 bits of e^(SCALE*s) via affine +
                    # int16 convert; bitcast back to f16 is free
                    nc.vector.tensor_scalar(
                        out=pt.bitcast(I16),
                        in0=stm,
                        scalar1=float(SCALE * SCHRAUD_A),
                        scalar2=SCHRAUD_B,
                        op0=mybir.AluOpType.mult,
                        op1=mybir.AluOpType.add,
                    )
                else:
                    nc.scalar.activation(
                        out=pt, in_=stm,
                        func=mybir.ActivationFunctionType.Exp, scale=SCALE,
                    )
            if prev is not None:
                ppr = prev[0]
                ptj = ptiles[prev][j]
                # one accumulation group per po bank: start zeroes the whole
                # 2KB zero region, so only the first write starts and only
                # the last stops
                for ib in range(4):
                    po = po_a if ib < 2 else po_b
                    for h in range(2):
                        r = (ib % 2) * 2 + h
                        nc.tensor.matmul(
                            po[:, r * 65:(r + 1) * 65],
                            ptj[:, h * 512 + ib * 128: h * 512 + (ib + 1) * 128],
                            vs[j][:, 2 * ppr + h, 0:65],
                            start=(j == 0 and r == 0),
                            stop=(j == NO - 1 and r == 3),
                        )
            credit[0] = min(credit[0] + (340 if cur and prev else
                                         700 if cur else 730), 2600)
            pop_fillers(widx * NO + j)
        if prev is not None:
            emit_drain(prev, po_a, po_b)
            del ptiles[prev]

    # -- emission schedule --
    # front: minimal deps for window (0,0), q/k interleaved per co so the
    # last matmul waits only the last x-chunk DMA. k is split so the first
    # 128-token chunk (all S^T(0,0,0) needs) lands before the k remainder.
    # front q/k psum groups borrow the po slots (idle until the first
    # deferred-PV window), decoupling the lead from the mm-slot rotation
    pk0 = pop_.tile([P, 128], F32, tag="po", name="pk0")
    pq0 = pop_.tile([P, 512], F32, tag="po", name="pq0")
    for co in range(CO):
        nc.tensor.matmul(
            pq0, ws[co][:, 0:128], xs[co][:, 0:512],
            start=(co == 0), stop=(co == CO - 1),
        )
        nc.tensor.matmul(
            pk0, ws[co][:, 128:256], xs[co][:, 0:128],
            start=(co == 0), stop=(co == CO - 1),
        )
    # q evacuates on the Act engine (idle through the lead) in parallel with
    # the k chunk's DVE copy, shortening the first-exp dependency chain
    nc.scalar.copy(out=qs[0][:, 0:512], in_=pq0)
    nc.vector.tensor_copy(out=ks_[0][:, 0:128], in_=pk0)
    boxk = {}
    for co in range(CO):
        # k n4=0 cols 128:512 as earliest fillers so S^T(0,0,0) (which only
        # needs the 128-col chunk above) is not queued behind them
        add_filler(215, 0, lambda co=co, box=boxk: qk_step_rest(0, co, box))
    for n4 in (1, 2, 3):
        # k chunk n4 feeds S^T(0,0,j=4*n4) at slot 4*n4
        add_qk_fillers(0, 1, n4, 4 * n4 - 2)
    for no in (0, 1, 2):
        # v[p3=0, no] feeds PV(prev=(0,0), j=no) at slot 16+no
        add_v_fillers(0, no, 13 + no)
    add_qk_fillers(0, 0, 1, 14)
    for no in range(3, NO):
        add_v_fillers(0, no, 13 + no)
    add_qk_fillers(0, 0, 2, 2 * NO - 2)
    add_qk_fillers(0, 0, 3, 3 * NO - 2)
    for pr in (1, 2):
        s0 = 4 * pr * NO
        add_qk_fillers(pr, 0, 0, s0 - 2)
        add_qk_fillers(pr, 1, 0, s0 - 2)
        for n4 in (1, 2, 3):
            add_qk_fillers(pr, 1, n4, s0 + 4 * n4 - 2)
        for n4 in (1, 2, 3):
            add_qk_fillers(pr, 0, n4, s0 + n4 * NO - 2)
        for no in range(NO):
            # v[pr, no] feeds PV(prev=(pr,0), j=no) at slot s0+16+no; due
            # inside the pair's own first window (quiet), keeping both the
            # congested pair-0 phase and the proj-carrying last windows free
            add_v_fillers(pr, no, s0 - 2 + no)

    wins = [(pr, i4) for pr in range(PAIRS) for i4 in range(NI4)]
    prev = None
    for widx, cur in enumerate(wins):
        window(widx, cur, prev, widx - 1)
        prev = cur
    window(len(wins), None, prev, len(wins) - 1)
    flush_fillers()


_NC_CACHE = {}


def build_bass():
    key = (SCHRAUD, SCHRAUD_B)
    if key in _NC_CACHE:
        return _NC_CACHE[key]
    nc = bass.Bass("TRN2")
    with tile.TileContext(nc) as tc:
        with ExitStack() as ctx:
            _emit(nc, tc, ctx)
    _split_multiwaits(nc)
    _NC_CACHE[key] = nc
    return nc


def make_in_maps(x, w_qkv, w_proj):
    x = np.asarray(x, dtype=np.float32)
    w_qkv = np.asarray(w_qkv, dtype=np.float32)
    w_proj = np.asarray(w_proj, dtype=np.float32)
    wq, wk, wv = w_qkv[0:C], w_qkv[C:2 * C], w_qkv[2 * C:3 * C]
    identd = np.eye(P, dtype=np.float16)
    in_maps = []
    for c in range(NCORES):
        b, g = divmod(c, 2)
        base = g * F
        cols = []
        for p3 in range(PAIRS):
            lo = base + p3 * 128
            cols.append(wq[lo:lo + 128])
            cols.append(wk[lo:lo + 128])
        cols.append(wv[base:base + F])
        wsel = np.concatenate(cols, axis=0)            # [1152, 768]
        wqd = np.ascontiguousarray(wsel.T.astype(np.float16)).reshape(CO, P, 3 * F)
        xcd = np.ascontiguousarray(x[b].T.astype(np.float16)).reshape(CO, P, N)
        wpd = np.stack(
            [
                np.ascontiguousarray(
                    w_proj[:, base + p3 * 128: base + (p3 + 1) * 128].T
                ).astype(np.float16)
                for p3 in range(PAIRS)
            ]
        )
        in_maps.append({"xcd": xcd, "wqd": wqd, "wpd": wpd, "identd": identd})
    return in_maps


def gather_output(parts, b_proj):
    """parts: 8 arrays [N, C] (head-group partials per core)."""
    outv = np.empty((B, N, C), np.float32)
    for b in range(B):
        outv[b] = parts[2 * b] + parts[2 * b + 1]
    outv += np.asarray(b_proj, dtype=np.float32)[None, None, :]
    return outv


def kernel(x, w_qkv, w_proj, b_proj, _run_kwargs=None):
    nc = build_bass()
    in_maps = make_in_maps(x, w_qkv, w_proj)
    res = bass_utils.run_bass_kernel_spmd(
        nc, in_maps, core_ids=list(range(NCORES)), **(_run_kwargs or {})
    )
    parts = [r["outd"] for r in res.results]
    outv = gather_output(parts, b_proj)
    if _run_kwargs is not None:
        kernel.last_results = res
    return outv

